# revision 25
# baseline (speedup 1.0000x reference)
"""RSCNN SA-module (MSG) forward, data-parallel across 8 Trainium2 NeuronCores.

Strategy (per spec sharding hint): pure data parallel over batch B=16 — each of
the 8 cores processes 2 point clouds end-to-end (ball query, grouping, RSConv);
FPS runs on the host (exact, bitwise-matching the reference) overlapped with
the input transfers. The small shared mapping/cr-conv parameters are
replicated and cached on device across calls. The three training-mode
BatchNorms need global-batch statistics, so per-device moments are combined
with cross-device pmean collectives — the only cross-core communication.

Transfer optimizations (the axon tunnel is ~55 MB/s with ~80 ms round-trip
latency, and every extra jit output costs a ~92 ms round trip): features are
shipped as bf16, the output is quantized on device to uint8 with per-(cloud,
channel) scales packed into two trailing columns of the SAME array (single
output, single fetch), and the large input puts are issued asynchronously
BEFORE the host FPS so wire time hides behind FPS compute. End-to-end
rel-err ~5e-3 against the f32 reference (gate is 2e-2).
"""

import numpy as np

B, N, NPOINT = 16, 4096, 1024
C_FEAT = 64
RADII = (0.1, 0.2)
NSAMPLES = (32, 64)
C_IN = C_FEAT + 3
C_OUT = 128
C_MID = C_OUT // 4
EPS = 1e-5

_W_ORDER = ["w_map1", "b_map1", "w_map2", "b_map2", "g_map", "be_map",
            "g_rs", "be_rs", "w_cr", "b_cr", "g_cr", "be_cr"]


def _build():
    import jax
    import jax.numpy as jnp
    try:
        from jax import shard_map
    except ImportError:
        from jax.experimental.shard_map import shard_map
    from jax.sharding import Mesh, NamedSharding, PartitionSpec as P

    devs = jax.devices()[:8]
    mesh = Mesh(np.array(devs), ("x",))

    def gather(pts, idx):
        return jax.vmap(lambda p, i: p[i])(pts, idx)

    def ball_query(xyz, new_xyz, radius, nsample):
        # First-nsample-in-order points within radius, padded with the first
        # hit. Dense compare+count — a matmul-heavy variant measured the same
        # speed (the exec is dispatch-overhead-bound) but intermittently
        # crashed the NeuronCore (NRT_EXEC_UNIT_UNRECOVERABLE), so the
        # boring formulation stays.
        Nn = xyz.shape[1]
        # d2 via the dot-product identity — one batched matmul instead of a
        # (b,M,N,3) f32 intermediate (~200 MB/core). Boundary rounding can
        # differ from the reference's sum-of-squares by ~1 ulp: a handful of
        # hit flips batch-wide, absorbed by the error budget.
        mn = jnp.einsum("bmc,bnc->bmn", new_xyz, xyz,
                        preferred_element_type=jnp.float32)
        m2 = jnp.sum(new_xyz * new_xyz, -1)
        n2 = jnp.sum(xyz * xyz, -1)
        d2 = (m2[:, :, None] - 2.0 * mn) + n2[:, None, :]
        hit = d2 < radius * radius
        rank = jnp.cumsum(hit.astype(jnp.int16), axis=-1)        # (b, M, N)
        # Clamping rank at nsample+1 keeps every comparison below unchanged
        # and lets the M*N*S compare volume run in int8.
        rank8 = jnp.minimum(rank, nsample + 1).astype(jnp.int8)
        tgt = jnp.arange(1, nsample + 1, dtype=jnp.int8)
        # index of the s-th in-order hit = #{n : rank[n] < s+1} (rank is
        # nondecreasing); equals Nn when fewer than s+1 hits exist (then
        # padded with the first hit).
        parts = []
        for m0 in range(0, rank8.shape[1], 256):
            rc = rank8[:, m0:m0 + 256, :, None]                  # (b,256,N,1)
            cnt = jnp.sum(rc < tgt, axis=2, dtype=jnp.int32)
            parts.append(cnt)
        idx = jnp.concatenate(parts, axis=1)                     # (b, M, S)
        first = idx[..., :1]
        return jnp.where(idx >= Nn, first, idx)

    def pconv2d(x, w, b):
        return jnp.einsum("bims,oi->boms", x, w) + b[None, :, None, None]

    def pconv1d(x, w, b):
        return jnp.einsum("bim,oi->bom", x, w) + b[None, :, None]

    def bn_global(x, g, b, axes):
        # channel-last BN with global (cross-device) moments
        m_loc = jnp.mean(x, axes, keepdims=True)
        m2_loc = jnp.mean(x * x, axes, keepdims=True)
        m = jax.lax.pmean(m_loc, "x")
        m2 = jax.lax.pmean(m2_loc, "x")
        v = m2 - m * m
        return (x - m) / jnp.sqrt(v + EPS) * g + b

    def rsconv(h, x, w1, b1, w2, b2, g_map, be_map, g_rs, be_rs,
               w_cr, b_cr, g_cr, be_cr):
        # h: (b,M,S,10) geometry channels; x: (b,M,S,67) rel-xyz + features
        h = jnp.einsum("bmsi,oi->bmso", h, w1) + b1
        h = jax.nn.relu(bn_global(h, g_map, be_map, (0, 1, 2)))
        h = jnp.einsum("bmsi,oi->bmso", h, w2) + b2
        y = jax.nn.relu(bn_global(h * x, g_rs, be_rs, (0, 1, 2)))
        y = jnp.max(y, axis=2)                                   # (b,M,67)
        y = jnp.einsum("bmi,oi->bmo", y, w_cr) + b_cr
        return jax.nn.relu(bn_global(y, g_cr, be_cr, (0, 1)))    # (b,M,128)

    def fwd(xyz, features, fidx, *ws):
        new_xyz = gather(xyz, fidx)
        outs = []
        for radius, nsample in zip(RADII, NSAMPLES):
            idx = ball_query(xyz, new_xyz, radius, nsample)
            gx = gather(xyz, idx)                                # (b,M,S,3)
            rel = gx - new_xyz[:, :, None, :]
            gf = gather(features, idx).astype(jnp.float32)       # bf16 gather
            dist = jnp.sqrt(jnp.sum(rel * rel, -1, keepdims=True) + 1e-12)
            ctr = jnp.broadcast_to(gx[:, :, :1, :], gx.shape)    # first neighbor
            h = jnp.concatenate([dist, ctr, gx, rel], -1)        # (b,M,S,10)
            x = jnp.concatenate([rel, gf], -1)                   # (b,M,S,67)
            outs.append(rsconv(h, x, *ws))
        out = jnp.concatenate(outs, axis=2)                      # (b,M,256), >=0
        # Per-(cloud, channel) uint8 quantization to shrink the output fetch
        # over the slow tunnel (values are post-relu, so non-negative).
        amax = jnp.max(out, axis=1)                              # (b,256)
        scale = jnp.where(amax > 0, amax / 255.0, 1.0)
        q = jnp.round(out / scale[:, None, :]).astype(jnp.uint8)
        q = q.transpose(0, 2, 1)                                 # (b,256,M)
        # Encode the f32 scale into two uint8 columns (mantissa/exponent) so
        # the kernel has a SINGLE output — each extra output costs a ~92 ms
        # round-trip on the axon PJRT client.
        e = jnp.floor(jnp.log2(scale))
        mant = scale * jnp.exp2(-e)                              # [1,2)
        m8 = jnp.clip(jnp.round((mant - 1.0) * 255.0), 0, 255).astype(jnp.uint8)
        e8 = jnp.clip(e + 128.0, 0, 255).astype(jnp.uint8)
        return jnp.concatenate(
            [q, m8[:, :, None], e8[:, :, None]], axis=2)         # (b,256,M+2)

    in_specs = (P("x"), P("x"), P("x")) + (P(),) * 12
    out_specs = P("x")
    try:
        fn = shard_map(fwd, mesh=mesh, in_specs=in_specs,
                       out_specs=out_specs, check_vma=False)
    except TypeError:
        fn = shard_map(fwd, mesh=mesh, in_specs=in_specs,
                       out_specs=out_specs, check_rep=False)
    fn = jax.jit(fn)
    return jax, mesh, NamedSharding, P, fn


def _fps_host(xyz):
    """Exact FPS for all clouds, batched over B, bitwise-matching the fp32
    reference (same per-pair op order (dx*dx+dy*dy)+dz*dz, f32 throughout,
    argmax first-index tie-break)."""
    B_, N_, _ = xyz.shape
    x = np.ascontiguousarray(xyz[:, :, 0])
    y = np.ascontiguousarray(xyz[:, :, 1])
    z = np.ascontiguousarray(xyz[:, :, 2])
    dists = np.full((B_, N_), 1e10, np.float32)
    out = np.zeros((B_, NPOINT), np.int32)
    last = np.zeros(B_, np.int64)
    ar = np.arange(B_)
    dx = np.empty((B_, N_), np.float32)
    dy = np.empty_like(dx)
    dz = np.empty_like(dx)
    dd = np.empty_like(dx)
    t = np.empty_like(dx)
    for i in range(1, NPOINT):
        px = x[ar, last][:, None]
        py = y[ar, last][:, None]
        pz = z[ar, last][:, None]
        np.subtract(x, px, out=dx)
        np.subtract(y, py, out=dy)
        np.subtract(z, pz, out=dz)
        np.multiply(dx, dx, out=dd)
        np.multiply(dy, dy, out=t)
        np.add(dd, t, out=dd)
        np.multiply(dz, dz, out=t)
        np.add(dd, t, out=dd)
        np.minimum(dists, dd, out=dists)
        last = np.argmax(dists, axis=1)
        out[:, i] = last
    return out


_STATE = {}


def kernel(**inputs):
    import ml_dtypes

    if "fn" not in _STATE:
        jax, mesh, NS, P, fn = _build()
        _STATE.update(jax=jax, mesh=mesh, NS=NS, P=P, fn=fn)
    jax = _STATE["jax"]
    mesh, NS, P, fn = _STATE["mesh"], _STATE["NS"], _STATE["P"], _STATE["fn"]

    shard = NS(mesh, P("x"))
    rep = NS(mesh, P())

    xyz_np = np.ascontiguousarray(np.asarray(inputs["xyz"], np.float32))
    feat_np = np.asarray(inputs["features"], np.float32)

    # Issue the big input puts asynchronously, then hide host FPS behind them.
    d_xyz = jax.device_put(xyz_np, shard)
    d_feat = jax.device_put(feat_np.astype(ml_dtypes.bfloat16), shard)

    # Replicated weights: transferred once, cached on device across calls.
    w_np = [np.asarray(inputs[n], np.float32) for n in _W_ORDER]
    cached = _STATE.get("w_cache")
    if cached is None or not all(
            np.array_equal(a, b) for a, b in zip(w_np, cached[0])):
        d_ws = [jax.device_put(w, rep) for w in w_np]
        _STATE["w_cache"] = (w_np, d_ws)
    d_ws = _STATE["w_cache"][1]

    fidx = _fps_host(xyz_np)
    d_fidx = jax.device_put(fidx, shard)

    try:
        packed = np.asarray(fn(d_xyz, d_feat, d_fidx, *d_ws))    # (B,256,M+2)
    except Exception:
        # Transient NRT device faults recover on the next execution; retry
        # once with freshly-placed inputs.
        d_xyz = jax.device_put(xyz_np, shard)
        d_feat = jax.device_put(feat_np.astype(ml_dtypes.bfloat16), shard)
        d_fidx = jax.device_put(fidx, shard)
        d_ws = [jax.device_put(w, rep) for w in w_np]
        _STATE["w_cache"] = (w_np, d_ws)
        packed = np.asarray(fn(d_xyz, d_feat, d_fidx, *d_ws))
    q = packed[:, :, :NPOINT]
    m8 = packed[:, :, NPOINT].astype(np.float32)
    e8 = packed[:, :, NPOINT + 1].astype(np.float32)
    scale = (1.0 + m8 / 255.0) * np.exp2(e8 - 128.0)
    return q.astype(np.float32) * scale[:, :, None]


# revision 26
# speedup vs baseline: 1.1656x; 1.1656x over previous
"""RSCNN SA-module (MSG) forward, data-parallel across 8 Trainium2 NeuronCores.

Strategy (per spec sharding hint): pure data parallel over batch B=16 — each of
the 8 cores processes 2 point clouds end-to-end (ball query, grouping, RSConv);
FPS runs on the host (exact, bitwise-matching the reference) overlapped with
the input transfers. The small shared mapping/cr-conv parameters are
replicated and cached on device across calls. The three training-mode
BatchNorms need global-batch statistics, so per-device moments are combined
with cross-device pmean collectives — the only cross-core communication.

Transfer optimizations (the axon tunnel is ~55 MB/s with ~80 ms round-trip
latency, and every extra jit output costs a ~92 ms round trip): features are
shipped as bf16, the output is quantized on device to uint8 with per-(cloud,
channel) scales packed into two trailing columns of the SAME array (single
output, single fetch), and the large input puts are issued asynchronously
BEFORE the host FPS so wire time hides behind FPS compute. End-to-end
rel-err ~5e-3 against the f32 reference (gate is 2e-2).
"""

import numpy as np

B, N, NPOINT = 16, 4096, 1024
C_FEAT = 64
RADII = (0.1, 0.2)
NSAMPLES = (32, 64)
C_IN = C_FEAT + 3
C_OUT = 128
C_MID = C_OUT // 4
EPS = 1e-5

_W_ORDER = ["w_map1", "b_map1", "w_map2", "b_map2", "g_map", "be_map",
            "g_rs", "be_rs", "w_cr", "b_cr", "g_cr", "be_cr"]


def _build():
    import jax
    import jax.numpy as jnp
    try:
        from jax import shard_map
    except ImportError:
        from jax.experimental.shard_map import shard_map
    from jax.sharding import Mesh, NamedSharding, PartitionSpec as P

    devs = jax.devices()[:8]
    mesh = Mesh(np.array(devs), ("x",))

    def gather(pts, idx):
        return jax.vmap(lambda p, i: p[i])(pts, idx)

    def ball_query(xyz, new_xyz, radius, nsample):
        # First-nsample-in-order points within radius, padded with the first
        # hit. Dense compare+count — a matmul-heavy variant measured the same
        # speed (the exec is dispatch-overhead-bound) but intermittently
        # crashed the NeuronCore (NRT_EXEC_UNIT_UNRECOVERABLE), so the
        # boring formulation stays.
        Nn = xyz.shape[1]
        d2 = jnp.sum((new_xyz[:, :, None, :] - xyz[:, None, :, :]) ** 2, -1)
        hit = d2 < radius * radius
        rank = jnp.cumsum(hit.astype(jnp.int16), axis=-1)        # (b, M, N)
        tgt = jnp.arange(1, nsample + 1, dtype=jnp.int16)
        # index of the s-th in-order hit = #{n : rank[n] < s+1} (rank is
        # nondecreasing); equals Nn when fewer than s+1 hits exist (then
        # padded with the first hit).
        parts = []
        for m0 in range(0, rank.shape[1], 256):
            rc = rank[:, m0:m0 + 256, :, None]                   # (b,256,N,1)
            cnt = jnp.sum((rc < tgt).astype(jnp.int16), axis=2)
            parts.append(cnt.astype(jnp.int32))
        idx = jnp.concatenate(parts, axis=1)                     # (b, M, S)
        first = idx[..., :1]
        return jnp.where(idx >= Nn, first, idx)

    def pconv2d(x, w, b):
        return jnp.einsum("bims,oi->boms", x, w) + b[None, :, None, None]

    def pconv1d(x, w, b):
        return jnp.einsum("bim,oi->bom", x, w) + b[None, :, None]

    def bn_global(x, g, b, axes):
        # channel-last BN with global (cross-device) moments
        m_loc = jnp.mean(x, axes, keepdims=True)
        m2_loc = jnp.mean(x * x, axes, keepdims=True)
        m = jax.lax.pmean(m_loc, "x")
        m2 = jax.lax.pmean(m2_loc, "x")
        v = m2 - m * m
        return (x - m) / jnp.sqrt(v + EPS) * g + b

    def rsconv(h, x, w1, b1, w2, b2, g_map, be_map, g_rs, be_rs,
               w_cr, b_cr, g_cr, be_cr):
        # h: (b,M,S,10) geometry channels; x: (b,M,S,67) rel-xyz + features
        h = jnp.einsum("bmsi,oi->bmso", h, w1) + b1
        h = jax.nn.relu(bn_global(h, g_map, be_map, (0, 1, 2)))
        h = jnp.einsum("bmsi,oi->bmso", h, w2) + b2
        y = jax.nn.relu(bn_global(h * x, g_rs, be_rs, (0, 1, 2)))
        y = jnp.max(y, axis=2)                                   # (b,M,67)
        y = jnp.einsum("bmi,oi->bmo", y, w_cr) + b_cr
        return jax.nn.relu(bn_global(y, g_cr, be_cr, (0, 1)))    # (b,M,128)

    def fwd(xyz, features, fidx, *ws):
        new_xyz = gather(xyz, fidx)
        outs = []
        for radius, nsample in zip(RADII, NSAMPLES):
            idx = ball_query(xyz, new_xyz, radius, nsample)
            gx = gather(xyz, idx)                                # (b,M,S,3)
            rel = gx - new_xyz[:, :, None, :]
            gf = gather(features, idx).astype(jnp.float32)       # bf16 gather
            dist = jnp.sqrt(jnp.sum(rel * rel, -1, keepdims=True) + 1e-12)
            ctr = jnp.broadcast_to(gx[:, :, :1, :], gx.shape)    # first neighbor
            h = jnp.concatenate([dist, ctr, gx, rel], -1)        # (b,M,S,10)
            x = jnp.concatenate([rel, gf], -1)                   # (b,M,S,67)
            outs.append(rsconv(h, x, *ws))
        out = jnp.concatenate(outs, axis=2)                      # (b,M,256), >=0
        # Per-(cloud, channel) uint8 quantization to shrink the output fetch
        # over the slow tunnel (values are post-relu, so non-negative).
        amax = jnp.max(out, axis=1)                              # (b,256)
        scale = jnp.where(amax > 0, amax / 255.0, 1.0)
        q = jnp.round(out / scale[:, None, :]).astype(jnp.uint8)
        q = q.transpose(0, 2, 1)                                 # (b,256,M)
        # Encode the f32 scale into two uint8 columns (mantissa/exponent) so
        # the kernel has a SINGLE output — each extra output costs a ~92 ms
        # round-trip on the axon PJRT client.
        e = jnp.floor(jnp.log2(scale))
        mant = scale * jnp.exp2(-e)                              # [1,2)
        m8 = jnp.clip(jnp.round((mant - 1.0) * 255.0), 0, 255).astype(jnp.uint8)
        e8 = jnp.clip(e + 128.0, 0, 255).astype(jnp.uint8)
        return jnp.concatenate(
            [q, m8[:, :, None], e8[:, :, None]], axis=2)         # (b,256,M+2)

    in_specs = (P("x"), P("x"), P("x")) + (P(),) * 12
    out_specs = P("x")
    try:
        fn = shard_map(fwd, mesh=mesh, in_specs=in_specs,
                       out_specs=out_specs, check_vma=False)
    except TypeError:
        fn = shard_map(fwd, mesh=mesh, in_specs=in_specs,
                       out_specs=out_specs, check_rep=False)
    fn = jax.jit(fn)
    return jax, mesh, NamedSharding, P, fn


def _fps_host(xyz):
    """Exact FPS for all clouds, batched over B, bitwise-matching the fp32
    reference (same per-pair op order (dx*dx+dy*dy)+dz*dz, f32 throughout,
    argmax first-index tie-break)."""
    B_, N_, _ = xyz.shape
    x = np.ascontiguousarray(xyz[:, :, 0])
    y = np.ascontiguousarray(xyz[:, :, 1])
    z = np.ascontiguousarray(xyz[:, :, 2])
    dists = np.full((B_, N_), 1e10, np.float32)
    out = np.zeros((B_, NPOINT), np.int32)
    last = np.zeros(B_, np.int64)
    ar = np.arange(B_)
    dx = np.empty((B_, N_), np.float32)
    dy = np.empty_like(dx)
    dz = np.empty_like(dx)
    dd = np.empty_like(dx)
    t = np.empty_like(dx)
    for i in range(1, NPOINT):
        px = x[ar, last][:, None]
        py = y[ar, last][:, None]
        pz = z[ar, last][:, None]
        np.subtract(x, px, out=dx)
        np.subtract(y, py, out=dy)
        np.subtract(z, pz, out=dz)
        np.multiply(dx, dx, out=dd)
        np.multiply(dy, dy, out=t)
        np.add(dd, t, out=dd)
        np.multiply(dz, dz, out=t)
        np.add(dd, t, out=dd)
        np.minimum(dists, dd, out=dists)
        last = np.argmax(dists, axis=1)
        out[:, i] = last
    return out


_STATE = {}


def kernel(**inputs):
    import ml_dtypes

    if "fn" not in _STATE:
        jax, mesh, NS, P, fn = _build()
        _STATE.update(jax=jax, mesh=mesh, NS=NS, P=P, fn=fn)
    jax = _STATE["jax"]
    mesh, NS, P, fn = _STATE["mesh"], _STATE["NS"], _STATE["P"], _STATE["fn"]

    shard = NS(mesh, P("x"))
    rep = NS(mesh, P())

    xyz_np = np.ascontiguousarray(np.asarray(inputs["xyz"], np.float32))
    feat_np = np.asarray(inputs["features"], np.float32)

    # Issue the big input puts asynchronously, then hide host FPS behind them.
    d_xyz = jax.device_put(xyz_np, shard)
    d_feat = jax.device_put(feat_np.astype(ml_dtypes.bfloat16), shard)

    # Replicated weights: transferred once, cached on device across calls.
    w_np = [np.asarray(inputs[n], np.float32) for n in _W_ORDER]
    cached = _STATE.get("w_cache")
    if cached is None or not all(
            np.array_equal(a, b) for a, b in zip(w_np, cached[0])):
        d_ws = [jax.device_put(w, rep) for w in w_np]
        _STATE["w_cache"] = (w_np, d_ws)
    d_ws = _STATE["w_cache"][1]

    fidx = _fps_host(xyz_np)
    d_fidx = jax.device_put(fidx, shard)

    try:
        packed = np.asarray(fn(d_xyz, d_feat, d_fidx, *d_ws))    # (B,256,M+2)
    except Exception:
        # Transient NRT device faults recover on the next execution; retry
        # once with freshly-placed inputs.
        d_xyz = jax.device_put(xyz_np, shard)
        d_feat = jax.device_put(feat_np.astype(ml_dtypes.bfloat16), shard)
        d_fidx = jax.device_put(fidx, shard)
        d_ws = [jax.device_put(w, rep) for w in w_np]
        _STATE["w_cache"] = (w_np, d_ws)
        packed = np.asarray(fn(d_xyz, d_feat, d_fidx, *d_ws))
    q = packed[:, :, :NPOINT]
    m8 = packed[:, :, NPOINT].astype(np.float32)
    e8 = packed[:, :, NPOINT + 1].astype(np.float32)
    scale = (1.0 + m8 / 255.0) * np.exp2(e8 - 128.0)
    return q.astype(np.float32) * scale[:, :, None]
